# revision 48
# baseline (speedup 1.0000x reference)
"""RGCN (2-layer, basis-decomposition) Trainium2 kernel.

Strategy (8 NeuronCores, SPMD), v4:
  - Edges sorted by destination; nodes are packed per-core into 32-node
    blocks (two 128-edge tiles per block).  Packing runs twice: round 1
    fixes each node's position parity, round 2 re-packs with per-block
    caps (<=16 even/odd-parity nodes, <=128 even/odd-source-parity
    edges) so every tile holds edges of a single source-parity class.
  - Structure: page = 32 tiles = 16 blocks = 4 groups = 1 tail quad.
    Within a page, tiles 0..15 are the even-parity tiles of its 16
    blocks, tiles 16..31 the odd ones.  NBLK is padded to 16*NPAGE with
    empty blocks.
  - Layer 1 does NO device-side gather: the host pre-materializes x[src]
    per edge slot (xedge pages, streamed with big sequential DMAs) and
    folds x@root1+bias1 into a per-node rootterm.  1/deg is folded into
    the per-edge coefficients.
  - Scatter+segment-sum run on the PE via "weighted one-hot" matmuls.
    Tails are computed transposed, one PSUM bank per page-quad: 4 basis
    matmuls (stationary basis, moving S); layer 2 adds per-group root
    matmuls (stationary root2|bias2, moving h^T|ones) into the same
    accumulation.  h^T lives in a persistent SBUF buffer; pair-layout h
    rows for the gather table are produced with DMA transposes on the
    scalar queue.  Layer-1 tails are emitted one page late so their
    dependency chains never head-of-line block the streaming queues.
    Layer-2 output is written transposed (host flips back).
  - h is exchanged via NCHUNK chunked AllGathers issued as chunks of
    pages complete, overlapping the collective with layer-1 compute.
    The replicated table is chunk-major: [chunk][core][pair rows].
  - Layer 2 gathers 256-B half rows (elem_step=512 B) with the gpsimd
    dma_gather ucode, two prepare_only preps per page (even-tile slots
    read the even half, odd-tile the odd half).  Descriptor generation
    for the first PREFETCH pages runs during layer 1, interleaved with
    the collective triggers on the gpsimd queue; trigger_dma fires the
    prepared rings once the table is ready.
"""

import math

import numpy as np
import ml_dtypes

import concourse.bacc as bacc
import concourse.bass as bass
import concourse.mybir as mybir
import concourse.tile as tile
from concourse.bass_utils import run_bass_kernel_spmd

F32 = mybir.dt.float32
BF16 = mybir.dt.bfloat16
I16 = mybir.dt.int16
AF = mybir.ActivationFunctionType
ALU = mybir.AluOpType
BF = ml_dtypes.bfloat16

M = 8            # cores
BLK = 32         # nodes per scatter block
GRP = 128        # nodes per output group (4 blocks)
QUAD = 4         # groups per tail quad (== groups per page)
TPE = 128        # edges per tile
G = 32           # tiles per page
CHK = 8          # tiles per wonehot build chunk
DW = 128         # padded table row width (256 B in bf16)
NCHUNK = 4       # allgather chunks
PREFETCH = 2     # gather pages whose desc-gen runs during layer 1
PREPS_BEFORE_CC = [1, 1, 0, 0]  # prefetch pages emitted before each cc
USE_PREP_TRIGGER = False  # prep/trigger gather pipeline (corrupts on this HW)
DEFER_TAILS = True  # deferred layer-1 tail emission


def _expand(ap, free_dims, col_offset=0):
    """AP with the partition dim kept and explicit [step, count] free dims."""
    base = ap.ap
    return bass.AP(
        ap.tensor,
        ap.offset + col_offset,
        [list(base[0])] + [list(d) for d in free_dims],
    )


def _pack(bounds, deg_e, deg_o, par0, caps_nodes):
    """Greedy packing with deferred-node lookahead honoring per-block caps.
    Returns (blk_of, j_of, nblk_core).  par0 None for round 1."""
    N = len(deg_e)
    blk_of = np.empty(N, dtype=np.int64)
    j_of = np.empty(N, dtype=np.int64)
    nblk_core = []
    for m in range(len(bounds) - 1):
        lo, hi = bounds[m], bounds[m + 1]
        b = 0
        nE = nO = ceE = ceO = 0
        deferred = []

        def fits(n):
            dE, dO = int(deg_e[n]), int(deg_o[n])
            if par0 is None:
                return (nE + 1 <= 2 * caps_nodes[0]
                        and ceE + dE + dO <= 2 * TPE)
            pn = int(par0[n])
            return (ceE + dE <= TPE and ceO + dO <= TPE
                    and (pn == 1 or nE + 1 <= caps_nodes[0])
                    and (pn == 0 or nO + 1 <= caps_nodes[1]))

        def place(n):
            nonlocal nE, nO, ceE, ceO
            dE, dO = int(deg_e[n]), int(deg_o[n])
            blk_of[n] = b
            if par0 is None:
                j_of[n] = nE
                nE += 1
                ceE += dE + dO
            else:
                if int(par0[n]) == 0:
                    j_of[n] = 2 * nE
                    nE += 1
                else:
                    j_of[n] = 2 * nO + 1
                    nO += 1
                ceE += dE
                ceO += dO

        n = lo
        while n < hi or deferred:
            placed = False
            for i, dn in enumerate(deferred):
                if fits(dn):
                    place(dn)
                    deferred.pop(i)
                    placed = True
                    break
            if placed:
                continue
            if n < hi:
                if fits(n):
                    place(n)
                else:
                    deferred.append(n)
                n += 1
            else:
                b += 1
                nE = nO = ceE = ceO = 0
        nblk_core.append(b + 1)
    return blk_of, j_of, nblk_core


def _prep(N, D, edge_index, edge_type, edge_norm, att1, att2,
          entity, root1, bias1):
    """Host-side graph preprocessing. Returns per-core arrays + structure."""
    src = np.asarray(edge_index[0], dtype=np.int64)
    dst = np.asarray(edge_index[1], dtype=np.int64)
    et = np.asarray(edge_type[:, 0], dtype=np.int64)
    norm = np.asarray(edge_norm, dtype=np.float32)
    E = len(src)

    order = np.argsort(dst, kind="stable")
    src_s, dst_s, et_s, norm_s = src[order], dst[order], et[order], norm[order]

    deg = np.bincount(dst, minlength=N).astype(np.int64)
    inv_deg = (1.0 / np.maximum(deg, 1.0)).astype(np.float32)

    # contiguous node ranges with ~equal edge counts
    cum = np.cumsum(deg)
    bounds = [0]
    for m in range(1, M):
        bounds.append(int(np.searchsorted(cum, m * E // M)))
    bounds.append(N)

    # round 1: parity-blind packing -> fixes node position parity
    zeros = np.zeros(N, dtype=np.int64)
    blk1, j1, _ = _pack(bounds, deg, zeros, None, (16, 16))
    par0 = ((blk1 * BLK + j1) % 2).astype(np.int64)

    # per-dst-node counts of even/odd source-parity edges
    esp = par0[src_s]
    deg_e = np.bincount(dst_s[esp == 0], minlength=N).astype(np.int64)
    deg_o = np.bincount(dst_s[esp == 1], minlength=N).astype(np.int64)

    # round 2: parity-preserving packing with per-parity caps
    blk_of, j_of, nblk_core = _pack(bounds, deg_e, deg_o, par0, (16, 16))
    assert np.all((blk_of * BLK + j_of) % 2 == par0)

    own = np.empty(N, dtype=np.int64)
    for m in range(M):
        own[bounds[m]:bounds[m + 1]] = m

    # pad NBLK so page = 16 blocks exactly
    NPAGE = -(-max(nblk_core) // 16)
    NBLK = NPAGE * 16
    NGRP = NBLK // 4
    NPC = NGRP * GRP
    SEG = NPC + GRP
    NTAB = SEG * M
    assert NTAB // 2 <= 32768, NTAB
    TT = NPAGE * G

    # tile t: block t//2; even-parity tile then odd-parity tile per block
    tile_blocks = []
    for t in range(TT):
        tile_blocks.append((t // 2, t % 2 == 0, t % 2 == 1))

    # allgather chunk boundaries: pages [4, 4, 3, 3] pattern in groups
    pb = [round(i * NPAGE / NCHUNK) for i in range(NCHUNK + 1)]
    gb = [q * QUAD for q in pb]
    assert gb[-1] == NGRP and all(gb[i] < gb[i + 1] for i in range(NCHUNK))
    crows = [(gb[c + 1] - gb[c]) * (GRP // 2) for c in range(NCHUNK)]
    crows[-1] += (SEG - NPC) // 2
    cbase = np.zeros(NCHUNK, dtype=np.int64)
    cbase[1:] = np.cumsum([M * r for r in crows])[:-1]

    pos = blk_of * BLK + j_of                      # node -> position
    # node -> replicated-table pair row (chunk-major layout)
    pchunk = np.minimum(
        np.searchsorted(np.array(gb[1:]) * (GRP // 2), pos // 2, side="right"),
        NCHUNK - 1)
    pairrow = cbase[pchunk] + own * np.array(crows)[pchunk] \
        + pos // 2 - np.array([gb[c] * (GRP // 2) for c in range(NCHUNK)])[pchunk]
    assert np.all((pos % 2) == par0)

    # per-edge slot: block b, source-parity e -> tile 2b + e
    ecore = own[dst_s]
    eblk = blk_of[dst_s]
    etile = eblk * 2 + esp
    cellid = ecore * TT + etile
    cell_starts = np.zeros(M * TT + 1, dtype=np.int64)
    cell_starts[1:] = np.cumsum(np.bincount(cellid, minlength=M * TT))
    corder = np.argsort(cellid, kind="stable")
    within = np.empty(E, dtype=np.int64)
    within[corder] = np.arange(E) - cell_starts[cellid[corder]]
    assert within.max() < TPE
    slot = etile * TPE + within                    # within-core linear slot

    doff = j_of[dst_s].astype(np.float32)
    c1 = (np.asarray(att1, np.float32)[et_s]
          * (norm_s * inv_deg[dst_s])[:, None])
    c2 = (np.asarray(att2, np.float32)[et_s]
          * (norm_s * inv_deg[dst_s])[:, None])

    entity = np.asarray(entity, dtype=np.float32)
    ent_bf = entity.astype(BF)
    rootfull = entity @ np.asarray(root1, np.float32) + np.asarray(bias1, np.float32)

    def pack_idx(lin):
        """[TT*TPE] pair rows -> [TPE, NPAGE*256] int16 layout: per page,
        one 4096-idx call; idx for linear i at [i%16, i//16], replicated."""
        out = np.empty((TPE, NPAGE * 256), np.int16)
        for p in range(NPAGE):
            li = lin[p * G * TPE:(p + 1) * G * TPE]
            w = li.reshape(256, 16).T                # [16, 256]: [i%16, i//16]
            out[:, p * 256:(p + 1) * 256] = np.tile(w, (8, 1))
        return np.ascontiguousarray(out)

    def fieldize(vals, nf):
        """[TT*TPE, nf] slot-major -> [NPAGE, TPE, nf*G] field-major bf16."""
        vp = vals.reshape(NPAGE, G, TPE, nf).transpose(0, 2, 3, 1)
        return np.ascontiguousarray(vp.reshape(NPAGE, TPE, nf * G).astype(BF))

    xedges, idxps, metac1s, metac2s, roots = [], [], [], [], []
    node_ids, node_pos = [], []
    for m in range(M):
        sel = ecore == m
        sl = slot[sel]

        # layer-1 x[src] slot table, [NPAGE, 128, G*DW]
        xe = np.zeros((TT * TPE, DW), dtype=BF)
        xe[sl, 0:D] = ent_bf[src_s[sel]]
        xe = xe.reshape(NPAGE, G, TPE, DW).transpose(0, 2, 1, 3) \
            .reshape(NPAGE, TPE, G * DW)
        xedges.append(np.ascontiguousarray(xe))

        # layer-2 gather indices (pair rows); dummy slots -> this core's
        # zero tail row in the last chunk
        zrowp = int(cbase[-1] + m * crows[-1]
                    + (gb[-1] - gb[-2]) * (GRP // 2))
        rows = np.full(TT * TPE, zrowp, dtype=np.int64)
        rows[sl] = pairrow[src_s[sel]]
        idxps.append(pack_idx(rows.astype(np.int16)))

        # metac: field-major [da, c_b x4]
        f1 = np.zeros((TT * TPE, 5), dtype=np.float32)
        f2 = np.zeros((TT * TPE, 5), dtype=np.float32)
        f1[:, 0] = 99.0
        f2[:, 0] = 99.0
        f1[sl, 0] = doff[sel]
        f2[sl, 0] = doff[sel]
        f1[sl, 1:] = c1[sel]
        f2[sl, 1:] = c2[sel]
        metac1s.append(fieldize(f1, 5))
        metac2s.append(fieldize(f2, 5))

        # per-position layer-1 root term, transposed [D, NPC]
        nid = np.nonzero(own == m)[0]
        p = pos[nid]
        rt = np.zeros((NPC, D), dtype=np.float32)
        rt[p] = rootfull[nid]
        roots.append(np.ascontiguousarray(rt.T))
        node_ids.append(nid)
        node_pos.append(p)

    return dict(NPC=NPC, SEG=SEG, NTAB=NTAB, NBLK=NBLK, NGRP=NGRP,
                NPAGE=NPAGE, TT=TT, tile_blocks=tile_blocks,
                gb=gb, crows=crows, cbase=cbase,
                xedge=xedges, idxp=idxps, metac1=metac1s, metac2=metac2s,
                roottermT=roots, node_ids=node_ids, node_pos=node_pos)


def _woh_chunk(nc, wohp, iota_t, metat, c):
    """Build the weighted-one-hot rhs for CHK tiles of chunk c."""
    oh8 = wohp.tile([TPE, CHK * BLK], BF16, tag="oh")
    nc.vector.tensor_tensor(
        out=_expand(oh8[:], [[BLK, CHK], [1, BLK]]),
        in0=iota_t,
        in1=_expand(metat[:], [[1, CHK], [0, BLK]], col_offset=c * CHK),
        op=ALU.is_equal,
    )
    wohF = wohp.tile([TPE, CHK * 4 * BLK], BF16, tag="wohF")
    nc.vector.tensor_tensor(
        out=_expand(wohF[:], [[4 * BLK, CHK], [BLK, 4], [1, BLK]]),
        in0=_expand(oh8[:], [[BLK, CHK], [0, 4], [1, BLK]]),
        in1=_expand(metat[:], [[1, CHK], [G, 4], [0, BLK]],
                    col_offset=G + c * CHK),
        op=ALU.mult,
    )
    return wohF


def _flush_to_sbig4(nc, gq, psum_blk, sbig4):
    """psum [D, 4*GRP] (block-major) -> sbig4 cols (basis-major, quad-wide)."""
    for b in range(4):
        nc.scalar.copy(
            out=_expand(sbig4[:], [[BLK, 4], [1, BLK]],
                        col_offset=b * QUAD * GRP + gq * GRP),
            in_=_expand(psum_blk[:], [[4 * BLK, 4], [1, BLK]],
                        col_offset=b * BLK),
        )


def _layer1(tc, nc, pools, prm, D, xedge, metap, iota_t, bas_t,
            roottermT, hTbuf, hsl_chunks):
    """Generator: yields ('cc', chunk) after a chunk's last table write and
    ('prep', k) slots where gather desc-gen prefetch should be emitted."""
    meta, xp, gath, wohp, hp, sb4p, psp = pools
    NPAGE, tile_blocks = prm["NPAGE"], prm["tile_blocks"]
    NGRP, gb = prm["NGRP"], prm["gb"]

    def chunk_of(g):
        for c in range(NCHUNK):
            if g < gb[c + 1]:
                return c
        raise AssertionError

    def emit_tail(p):
        """Deferred tail for page p (groups 4p..4p+3)."""
        g0 = p * QUAD
        sbig4 = sbig4s.pop(p)
        pquad = psp.tile([D, QUAD * GRP], F32, tag="quad", bufs=2)
        for b in range(4):
            nc.tensor.matmul(
                out=pquad[:],
                lhsT=bas_t[:, b * D:(b + 1) * D],
                rhs=sbig4[:, b * QUAD * GRP:(b + 1) * QUAD * GRP],
                start=(b == 0), stop=(b == 3))
        rtt = rtts.pop(p)
        hq = hp.tile([D, QUAD * GRP], F32, tag="hq")
        nc.vector.tensor_tensor(out=hq[:], in0=pquad[:], in1=rtt[:],
                                op=ALU.add)
        nc.scalar.activation(
            out=hTbuf[0:D, g0 * GRP:(g0 + QUAD) * GRP],
            in_=hq[:], func=AF.Relu)
        for gg in range(g0, g0 + QUAD):
            htr = hp.tile([GRP, DW], BF16, tag="htr")
            nc.scalar.dma_start_transpose(
                out=htr[:], in_=hTbuf[:, gg * GRP:(gg + 1) * GRP])
            ck = chunk_of(gg)
            hc = hsl_chunks[ck]
            grel = gg - gb[ck]
            base = hc[:]
            dstap = bass.AP(
                base.tensor,
                base.offset + grel * (GRP // 2) * (2 * DW),
                [[2 * DW, GRP // 2], [DW, 2], [1, DW]])
            nc.scalar.dma_start(out=dstap, in_=htr[:])

    sbig4s = {}
    rtts = {}
    tcount = 0
    pending = None
    psum_blks = {}
    for q in range(NPAGE):
        metat = meta.tile([TPE, 5 * G], BF16, tag="metat1")
        nc.sync.dma_start(out=metat[:], in_=metap[q])
        xpage = xp.tile([TPE, G * DW], BF16, tag="xpage")
        nc.sync.dma_start(out=xpage[:], in_=xedge[q])
        sbig4s[q] = sb4p.tile([D, 4 * QUAD * GRP], BF16, tag="sbig4", name="sbig4")
        rtts[q] = hp.tile([D, QUAD * GRP], F32, tag="rtt", name="rtt")
        nc.sync.dma_start(
            out=rtts[q][:],
            in_=roottermT[:, q * QUAD * GRP:(q + 1) * QUAD * GRP])
        for c in range(G // CHK):
            wohF = _woh_chunk(nc, wohp, iota_t, metat, c)
            if pending is not None and c == 2:
                p = pending
                pending = None
                emit_tail(p)
                gend = (p + 1) * QUAD - 1
                ck = chunk_of(gend)
                if gend == gb[ck + 1] - 1:
                    yield ('cc', ck)
            for u in range(CHK):
                blk, fst, lst = tile_blocks[tcount]
                g, bi = blk // 4, blk % 4
                gq = g % QUAD
                if fst:
                    if bi == 0:
                        psum_blks[g] = psp.tile([D, 4 * GRP], F32, tag="blk",
                                                bufs=4, name="pblk")
                    pb = psum_blks[g]
                else:
                    pb = psum_blks[g]
                nc.tensor.matmul(
                    out=pb[:, bi * 4 * BLK:(bi + 1) * 4 * BLK],
                    lhsT=xpage[:, (c * CHK + u) * DW:(c * CHK + u) * DW + D],
                    rhs=wohF[:, u * 4 * BLK:(u + 1) * 4 * BLK],
                    start=fst, stop=lst)
                if lst and bi == 3:
                    _flush_to_sbig4(nc, gq, psum_blks.pop(g), sbig4s[q])
                    if gq == QUAD - 1:
                        if DEFER_TAILS:
                            pending = q
                        else:
                            emit_tail(q)
                            gend = (q + 1) * QUAD - 1
                            ck = chunk_of(gend)
                            if gend == gb[ck + 1] - 1:
                                yield ('cc', ck)
                tcount += 1
    if pending is not None:
        p = pending
        emit_tail(p)
        gend = (p + 1) * QUAD - 1
        ck = chunk_of(gend)
        if gend == gb[ck + 1] - 1:
            yield ('cc', ck)


def _layer2(tc, nc, pools, prm, D, idxall, metap, preps, iota_t, bas_t,
            rt_t, hTbuf, outT, emit_prep):
    meta, xp, gath, wohp, hp, sb4p, psp = pools
    NPAGE, tile_blocks = prm["NPAGE"], prm["tile_blocks"]

    tcount = 0
    psum_blks = {}
    for q in range(NPAGE):
        if USE_PREP_TRIGGER:
            if q == 0:
                nc.gpsimd.trigger_dma(count=None)  # fire prefetched pages
            elif q - 1 + PREFETCH < NPAGE:
                emit_prep(q - 1 + PREFETCH)
                nc.gpsimd.trigger_dma(count=None)
            gbuf = preps[q]
        else:
            gbuf = emit_prep(q)
        metat = meta.tile([TPE, 5 * G], BF16, tag="metat2")
        nc.sync.dma_start(out=metat[:], in_=metap[q])
        sbig4 = sb4p.tile([D, 4 * QUAD * GRP], BF16, tag="sbig4")
        for c in range(G // CHK):
            wohF = _woh_chunk(nc, wohp, iota_t, metat, c)
            for u in range(CHK):
                blk, fst, lst = tile_blocks[tcount]
                g, bi = blk // 4, blk % 4
                gq = g % QUAD
                if fst and bi == 0:
                    psum_blks[g] = psp.tile([D, 4 * GRP], F32, tag="blk",
                                            bufs=4, name="pblk")
                pb = psum_blks[g]
                toff = (tcount % 2) * DW  # tile parity half
                nc.tensor.matmul(
                    out=pb[:, bi * 4 * BLK:(bi + 1) * 4 * BLK],
                    lhsT=gbuf[:, c * CHK + u, toff:toff + D],
                    rhs=wohF[:, u * 4 * BLK:(u + 1) * 4 * BLK],
                    start=fst, stop=lst)
                if lst and bi == 3:
                    _flush_to_sbig4(nc, gq, psum_blks.pop(g), sbig4)
                    if gq == QUAD - 1:
                        g0 = q * QUAD
                        pquad = psp.tile([D, QUAD * GRP], F32, tag="quad",
                                         bufs=2)
                        for b in range(4):
                            nc.tensor.matmul(
                                out=pquad[:],
                                lhsT=bas_t[:, b * D:(b + 1) * D],
                                rhs=sbig4[:, b * QUAD * GRP:
                                          (b + 1) * QUAD * GRP],
                                start=(b == 0), stop=False)
                        for gg in range(QUAD):
                            # root2 augmented with a bias row; hTbuf row D
                            # is all-ones so bias2 is added per column
                            nc.tensor.matmul(
                                out=pquad[:, gg * GRP:(gg + 1) * GRP],
                                lhsT=rt_t,
                                rhs=hTbuf[0:D + 1,
                                          (g0 + gg) * GRP:(g0 + gg + 1) * GRP],
                                start=False, stop=(gg == QUAD - 1))
                        o3 = hp.tile([D, QUAD * GRP], F32, tag="o3")
                        nc.scalar.copy(out=o3[:], in_=pquad[:])
                        nc.sync.dma_start(
                            out=outT[:, g0 * GRP:(g0 + QUAD) * GRP],
                            in_=o3[:])
                tcount += 1


def _build(prm, D):
    NPC, SEG, NTAB = prm["NPC"], prm["SEG"], prm["NTAB"]
    NPAGE, NGRP, gb = prm["NPAGE"], prm["NGRP"], prm["gb"]
    nc = bacc.Bacc()

    xedge = nc.dram_tensor("xedge", [NPAGE, TPE, G * DW], BF16, kind="ExternalInput")
    idxp = nc.dram_tensor("idxp", [TPE, NPAGE * 2 * 128], I16, kind="ExternalInput")
    metac1 = nc.dram_tensor("metac1", [NPAGE, TPE, 5 * G], BF16, kind="ExternalInput")
    metac2 = nc.dram_tensor("metac2", [NPAGE, TPE, 5 * G], BF16, kind="ExternalInput")
    roottermT = nc.dram_tensor("roottermT", [D, NPC], F32, kind="ExternalInput")
    iota = nc.dram_tensor("iota", [TPE, CHK * BLK], BF16, kind="ExternalInput")
    bas1 = nc.dram_tensor("bas1", [D, 4 * D], BF16, kind="ExternalInput")
    bas2 = nc.dram_tensor("bas2", [D, 4 * D], BF16, kind="ExternalInput")
    rt2 = nc.dram_tensor("rt2", [D + 1, D], BF16, kind="ExternalInput")
    cpad = nc.dram_tensor("cpad", [TPE - D, NPC], BF16, kind="ExternalInput")
    outT = nc.dram_tensor("outT", [D, NPC], F32, kind="ExternalOutput")
    hdump = nc.dram_tensor("hdump", [TPE, NPC], BF16, kind="ExternalOutput")

    with tile.TileContext(nc) as tc:
        with (
            tc.tile_pool(name="const", bufs=1) as cst,
            tc.tile_pool(name="meta", bufs=3) as meta,
            tc.tile_pool(name="xp", bufs=2) as xp,
            tc.tile_pool(name="gath", bufs=3) as gath,
            tc.tile_pool(name="woh", bufs=3) as wohp,
            tc.tile_pool(name="hp", bufs=2) as hp,
            tc.tile_pool(name="sb4", bufs=2) as sb4p,
            tc.tile_pool(name="ps", bufs=1, space="PSUM") as psp,
            tc.tile_pool(name="dram", bufs=1, space="DRAM") as dramp,
        ):
            pools = (meta, xp, gath, wohp, hp, sb4p, psp)

            iota_t = cst.tile([TPE, CHK * BLK], BF16)
            nc.sync.dma_start(out=iota_t[:], in_=iota[:])
            bas1_t = cst.tile([D, 4 * D], BF16)
            nc.sync.dma_start(out=bas1_t[:], in_=bas1[:])
            bas2_t = cst.tile([D, 4 * D], BF16)
            nc.sync.dma_start(out=bas2_t[:], in_=bas2[:])
            rt2_t = cst.tile([D + 1, D], BF16)
            nc.sync.dma_start(out=rt2_t[:], in_=rt2[:])
            zed_t = cst.tile([GRP, 2 * DW], BF16)
            nc.gpsimd.memset(zed_t[:], 0.0)
            idxall = cst.tile([TPE, NPAGE * 2 * 128], I16)
            nc.sync.dma_start(out=idxall[:], in_=idxp[:])
            # hTbuf rows D..127: row D all-ones (bias row for the layer-2
            # root matmul), rest zeros -- DMAed from a host constant.
            hTbuf = cst.tile([TPE, NPC], BF16)
            nc.sync.dma_start(out=hTbuf[D:TPE, :], in_=cpad[:])

            # per-chunk local h slices (pair-row layout); last chunk carries
            # the zero tail rows (dummy gather target)
            hsl_chunks = []
            chunk_rows = []
            for ckk in range(NCHUNK):
                rows = (gb[ckk + 1] - gb[ckk]) * (GRP // 2)
                if ckk == NCHUNK - 1:
                    rows += (SEG - NPC) // 2
                t = dramp.tile([rows, 2 * DW], BF16, name=f"hslc{ckk}")
                hsl_chunks.append(t)
                chunk_rows.append(rows)

            hfull = dramp.tile([NTAB // 2, 2 * DW], BF16)

            # zero rows at the tail of the last chunk
            tail_rows = (SEG - NPC) // 2
            nc.sync.dma_start(
                out=hsl_chunks[-1][chunk_rows[-1] - tail_rows:chunk_rows[-1], :],
                in_=zed_t[0:tail_rows, :])

            dma_sem = nc.alloc_semaphore("swdge_gather")
            preps = {}

            def emit_prep(p):
                """Full-pair-row gather for page p (prep or inline)."""
                gbuf = gath.tile([TPE, G, 2 * DW], BF16, tag="gbuf")
                preps[p] = gbuf
                kw = {}
                if USE_PREP_TRIGGER:
                    kw = dict(prepare_only=True, sem=dma_sem)
                nc.gpsimd.dma_gather(
                    out_ap=gbuf[:], in_ap=hfull[:, :],
                    idxs_ap=idxall[:, p * 256:(p + 1) * 256],
                    num_idxs=G * TPE, num_idxs_reg=G * TPE,
                    elem_size=2 * DW, single_packet=False, **kw)
                return gbuf

            cbase = prm["cbase"]
            nprep = 0
            if USE_PREP_TRIGGER:
                for k in range(min(PREPS_BEFORE_CC[0], NPAGE - nprep)):
                    emit_prep(nprep)
                    nprep += 1
            ncc = 0
            for kind, ck in _layer1(tc, nc, pools, prm, D, xedge, metac1,
                                    iota_t[:], bas1_t, roottermT, hTbuf,
                                    hsl_chunks):
                assert kind == 'cc'
                rows = chunk_rows[ck]
                nc.gpsimd.collective_compute(
                    "AllGather",
                    ALU.bypass,
                    replica_groups=[list(range(M))],
                    ins=[hsl_chunks[ck][:]],
                    outs=[hfull[cbase[ck]:cbase[ck] + M * rows, :]],
                )
                ncc += 1
                if USE_PREP_TRIGGER and ncc < NCHUNK:
                    budget = PREPS_BEFORE_CC[ncc]
                    for k in range(min(budget,
                                       min(PREFETCH, NPAGE) - nprep)):
                        emit_prep(nprep)
                        nprep += 1
            while USE_PREP_TRIGGER and nprep < min(PREFETCH, NPAGE):
                emit_prep(nprep)
                nprep += 1

            _layer2(tc, nc, pools, prm, D, idxall, metac2, preps,
                    iota_t[:], bas2_t, rt2_t[:], hTbuf, outT, emit_prep)
            nc.sync.dma_start(out=hdump[:], in_=hTbuf[:])
    nc.compile()
    return nc


def kernel(entity, edge_index, edge_attr, edge_type, edge_norm,
           basis1, att1, root1, bias1, basis2, att2, root2, bias2):
    N, D = entity.shape
    entity = np.asarray(entity, dtype=np.float32)
    prm = _prep(N, D, np.asarray(edge_index), np.asarray(edge_type),
                np.asarray(edge_norm), np.asarray(att1), np.asarray(att2),
                entity, np.asarray(root1), np.asarray(bias1))
    NPC = prm["NPC"]

    iota_arr = np.tile(np.arange(BLK, dtype=np.float32), (TPE, CHK)).astype(BF)
    b1 = np.ascontiguousarray(
        np.asarray(basis1, np.float32).transpose(1, 0, 2).reshape(D, 4 * D)).astype(BF)
    b2 = np.ascontiguousarray(
        np.asarray(basis2, np.float32).transpose(1, 0, 2).reshape(D, 4 * D)).astype(BF)

    nc = _build(prm, D)

    cpad_arr = np.zeros((TPE - D, NPC), dtype=BF)
    cpad_arr[0, :] = 1.0

    in_maps = []
    for m in range(M):
        in_maps.append({
            "xedge": prm["xedge"][m],
            "idxp": prm["idxp"][m],
            "metac1": prm["metac1"][m],
            "metac2": prm["metac2"][m],
            "roottermT": prm["roottermT"][m],
            "iota": iota_arr,
            "bas1": b1,
            "bas2": b2,
            "rt2": np.concatenate(
                [np.asarray(root2, np.float32),
                 np.asarray(bias2, np.float32).reshape(1, D)]).astype(BF),
            "cpad": cpad_arr,
        })
    kwargs = {}
    if TRACE:
        kwargs = dict(trace=True, tmpdir=TRACE_DIR)
    res = run_bass_kernel_spmd(nc, in_maps, core_ids=list(range(M)), **kwargs)
    global LAST
    LAST = res
    out = np.empty((N, D), dtype=np.float32)
    for m in range(M):
        o = np.ascontiguousarray(res.results[m]["outT"].T)
        out[prm["node_ids"][m]] = o[prm["node_pos"][m]]
    return np.ascontiguousarray(out)


LAST = None
TRACE = False
TRACE_DIR = None


# revision 49
# speedup vs baseline: 1.0057x; 1.0057x over previous
"""RGCN (2-layer, basis-decomposition) Trainium2 kernel.

Strategy (8 NeuronCores, SPMD), v4:
  - Edges sorted by destination; nodes are packed per-core into 32-node
    blocks (two 128-edge tiles per block).  Packing runs twice: round 1
    fixes each node's position parity, round 2 re-packs with per-block
    caps (<=16 even/odd-parity nodes, <=128 even/odd-source-parity
    edges) so every tile holds edges of a single source-parity class.
  - Structure: page = 32 tiles = 16 blocks = 4 groups = 1 tail quad.
    Within a page, tiles 0..15 are the even-parity tiles of its 16
    blocks, tiles 16..31 the odd ones.  NBLK is padded to 16*NPAGE with
    empty blocks.
  - Layer 1 does NO device-side gather: the host pre-materializes x[src]
    per edge slot (xedge pages, streamed with big sequential DMAs) and
    folds x@root1+bias1 into a per-node rootterm.  1/deg is folded into
    the per-edge coefficients.
  - Scatter+segment-sum run on the PE via "weighted one-hot" matmuls.
    Tails are computed transposed, one PSUM bank per page-quad: 4 basis
    matmuls (stationary basis, moving S); layer 2 adds per-group root
    matmuls (stationary root2|bias2, moving h^T|ones) into the same
    accumulation.  h^T lives in a persistent SBUF buffer; pair-layout h
    rows for the gather table are produced with DMA transposes on the
    scalar queue.  Layer-1 tails are emitted one page late so their
    dependency chains never head-of-line block the streaming queues.
    Layer-2 output is written transposed (host flips back).
  - h is exchanged via NCHUNK chunked AllGathers issued as chunks of
    pages complete, overlapping the collective with layer-1 compute.
    The replicated table is chunk-major: [chunk][core][pair rows].
  - Layer 2 gathers 256-B half rows (elem_step=512 B) with the gpsimd
    dma_gather ucode, two prepare_only preps per page (even-tile slots
    read the even half, odd-tile the odd half).  Descriptor generation
    for the first PREFETCH pages runs during layer 1, interleaved with
    the collective triggers on the gpsimd queue; trigger_dma fires the
    prepared rings once the table is ready.
"""

import math

import numpy as np
import ml_dtypes

import concourse.bacc as bacc
import concourse.bass as bass
import concourse.mybir as mybir
import concourse.tile as tile
from concourse.bass_utils import run_bass_kernel_spmd

F32 = mybir.dt.float32
BF16 = mybir.dt.bfloat16
I16 = mybir.dt.int16
AF = mybir.ActivationFunctionType
ALU = mybir.AluOpType
BF = ml_dtypes.bfloat16

M = 8            # cores
BLK = 32         # nodes per scatter block
GRP = 128        # nodes per output group (4 blocks)
QUAD = 4         # groups per tail quad (== groups per page)
TPE = 128        # edges per tile
G = 32           # tiles per page
CHK = 8          # tiles per wonehot build chunk
DW = 128         # padded table row width (256 B in bf16)
NCHUNK = 4       # allgather chunks
PREFETCH = 2     # gather pages whose desc-gen runs during layer 1
PREPS_BEFORE_CC = [1, 1, 0, 0]  # prefetch pages emitted before each cc
USE_PREP_TRIGGER = False  # prep/trigger gather pipeline (corrupts on this HW)
DEFER_TAILS = True  # deferred layer-1 tail emission


def _expand(ap, free_dims, col_offset=0):
    """AP with the partition dim kept and explicit [step, count] free dims."""
    base = ap.ap
    return bass.AP(
        ap.tensor,
        ap.offset + col_offset,
        [list(base[0])] + [list(d) for d in free_dims],
    )


def _pack(bounds, deg_e, deg_o, par0, caps_nodes):
    """Greedy packing with deferred-node lookahead honoring per-block caps.
    Returns (blk_of, j_of, nblk_core).  par0 None for round 1."""
    N = len(deg_e)
    blk_of = np.empty(N, dtype=np.int64)
    j_of = np.empty(N, dtype=np.int64)
    nblk_core = []
    for m in range(len(bounds) - 1):
        lo, hi = bounds[m], bounds[m + 1]
        b = 0
        nE = nO = ceE = ceO = 0
        deferred = []

        def fits(n):
            dE, dO = int(deg_e[n]), int(deg_o[n])
            if par0 is None:
                return (nE + 1 <= 2 * caps_nodes[0]
                        and ceE + dE + dO <= 2 * TPE)
            pn = int(par0[n])
            return (ceE + dE <= TPE and ceO + dO <= TPE
                    and (pn == 1 or nE + 1 <= caps_nodes[0])
                    and (pn == 0 or nO + 1 <= caps_nodes[1]))

        def place(n):
            nonlocal nE, nO, ceE, ceO
            dE, dO = int(deg_e[n]), int(deg_o[n])
            blk_of[n] = b
            if par0 is None:
                j_of[n] = nE
                nE += 1
                ceE += dE + dO
            else:
                if int(par0[n]) == 0:
                    j_of[n] = 2 * nE
                    nE += 1
                else:
                    j_of[n] = 2 * nO + 1
                    nO += 1
                ceE += dE
                ceO += dO

        n = lo
        while n < hi or deferred:
            placed = False
            for i, dn in enumerate(deferred):
                if fits(dn):
                    place(dn)
                    deferred.pop(i)
                    placed = True
                    break
            if placed:
                continue
            if n < hi:
                if fits(n):
                    place(n)
                else:
                    deferred.append(n)
                n += 1
            else:
                b += 1
                nE = nO = ceE = ceO = 0
        nblk_core.append(b + 1)
    return blk_of, j_of, nblk_core


def _prep(N, D, edge_index, edge_type, edge_norm, att1, att2,
          entity, root1, bias1):
    """Host-side graph preprocessing. Returns per-core arrays + structure."""
    src = np.asarray(edge_index[0], dtype=np.int64)
    dst = np.asarray(edge_index[1], dtype=np.int64)
    et = np.asarray(edge_type[:, 0], dtype=np.int64)
    norm = np.asarray(edge_norm, dtype=np.float32)
    E = len(src)

    order = np.argsort(dst, kind="stable")
    src_s, dst_s, et_s, norm_s = src[order], dst[order], et[order], norm[order]

    deg = np.bincount(dst, minlength=N).astype(np.int64)
    inv_deg = (1.0 / np.maximum(deg, 1.0)).astype(np.float32)

    # contiguous node ranges with ~equal edge counts
    cum = np.cumsum(deg)
    bounds = [0]
    for m in range(1, M):
        bounds.append(int(np.searchsorted(cum, m * E // M)))
    bounds.append(N)

    # round 1: parity-blind packing -> fixes node position parity
    zeros = np.zeros(N, dtype=np.int64)
    blk1, j1, _ = _pack(bounds, deg, zeros, None, (16, 16))
    par0 = ((blk1 * BLK + j1) % 2).astype(np.int64)

    # per-dst-node counts of even/odd source-parity edges
    esp = par0[src_s]
    deg_e = np.bincount(dst_s[esp == 0], minlength=N).astype(np.int64)
    deg_o = np.bincount(dst_s[esp == 1], minlength=N).astype(np.int64)

    # round 2: parity-preserving packing with per-parity caps
    blk_of, j_of, nblk_core = _pack(bounds, deg_e, deg_o, par0, (16, 16))
    assert np.all((blk_of * BLK + j_of) % 2 == par0)

    own = np.empty(N, dtype=np.int64)
    for m in range(M):
        own[bounds[m]:bounds[m + 1]] = m

    # pad NBLK so page = 16 blocks exactly
    NPAGE = -(-max(nblk_core) // 16)
    NBLK = NPAGE * 16
    NGRP = NBLK // 4
    NPC = NGRP * GRP
    SEG = NPC + GRP
    NTAB = SEG * M
    assert NTAB // 2 <= 32768, NTAB
    TT = NPAGE * G

    # tile t: block t//2; even-parity tile then odd-parity tile per block
    tile_blocks = []
    for t in range(TT):
        tile_blocks.append((t // 2, t % 2 == 0, t % 2 == 1))

    # allgather chunk boundaries: pages [4, 4, 3, 3] pattern in groups
    pb = [round(i * NPAGE / NCHUNK) for i in range(NCHUNK + 1)]
    gb = [q * QUAD for q in pb]
    assert gb[-1] == NGRP and all(gb[i] < gb[i + 1] for i in range(NCHUNK))
    crows = [(gb[c + 1] - gb[c]) * (GRP // 2) for c in range(NCHUNK)]
    crows[-1] += (SEG - NPC) // 2
    cbase = np.zeros(NCHUNK, dtype=np.int64)
    cbase[1:] = np.cumsum([M * r for r in crows])[:-1]

    pos = blk_of * BLK + j_of                      # node -> position
    # node -> replicated-table pair row (chunk-major layout)
    pchunk = np.minimum(
        np.searchsorted(np.array(gb[1:]) * (GRP // 2), pos // 2, side="right"),
        NCHUNK - 1)
    pairrow = cbase[pchunk] + own * np.array(crows)[pchunk] \
        + pos // 2 - np.array([gb[c] * (GRP // 2) for c in range(NCHUNK)])[pchunk]
    assert np.all((pos % 2) == par0)

    # per-edge slot: block b, source-parity e -> tile 2b + e
    ecore = own[dst_s]
    eblk = blk_of[dst_s]
    etile = eblk * 2 + esp
    cellid = ecore * TT + etile
    cell_starts = np.zeros(M * TT + 1, dtype=np.int64)
    cell_starts[1:] = np.cumsum(np.bincount(cellid, minlength=M * TT))
    corder = np.argsort(cellid, kind="stable")
    within = np.empty(E, dtype=np.int64)
    within[corder] = np.arange(E) - cell_starts[cellid[corder]]
    assert within.max() < TPE
    slot = etile * TPE + within                    # within-core linear slot

    doff = j_of[dst_s].astype(np.float32)
    c1 = (np.asarray(att1, np.float32)[et_s]
          * (norm_s * inv_deg[dst_s])[:, None])
    c2 = (np.asarray(att2, np.float32)[et_s]
          * (norm_s * inv_deg[dst_s])[:, None])

    entity = np.asarray(entity, dtype=np.float32)
    ent_bf = entity.astype(BF)
    rootfull = entity @ np.asarray(root1, np.float32) + np.asarray(bias1, np.float32)

    def pack_idx(lin):
        """[TT*TPE] pair rows -> [TPE, NPAGE*256] int16: per page two
        2048-idx calls (even tiles then odd tiles); call-linear i covers
        tile 2*(i//128)+e row i%128; idx at [i%16, i//16], replicated."""
        out = np.empty((TPE, NPAGE * 256), np.int16)
        lp = lin.reshape(NPAGE, G, TPE)
        for p in range(NPAGE):
            for e in range(2):
                li = lp[p, e::2, :].reshape(-1)      # [2048]
                w = li.reshape(128, 16).T            # [16, 128]
                out[:, (p * 2 + e) * 128:(p * 2 + e + 1) * 128] = \
                    np.tile(w, (8, 1))
        return np.ascontiguousarray(out)

    def fieldize(vals, nf):
        """[TT*TPE, nf] slot-major -> [NPAGE, TPE, nf*G] field-major bf16."""
        vp = vals.reshape(NPAGE, G, TPE, nf).transpose(0, 2, 3, 1)
        return np.ascontiguousarray(vp.reshape(NPAGE, TPE, nf * G).astype(BF))

    xedges, idxps, metac1s, metac2s, roots = [], [], [], [], []
    node_ids, node_pos = [], []
    for m in range(M):
        sel = ecore == m
        sl = slot[sel]

        # layer-1 x[src] slot table, [NPAGE, 128, G*DW]
        xe = np.zeros((TT * TPE, DW), dtype=BF)
        xe[sl, 0:D] = ent_bf[src_s[sel]]
        xe = xe.reshape(NPAGE, G, TPE, DW).transpose(0, 2, 1, 3) \
            .reshape(NPAGE, TPE, G * DW)
        xedges.append(np.ascontiguousarray(xe))

        # layer-2 gather indices (pair rows); dummy slots -> this core's
        # zero tail row in the last chunk
        zrowp = int(cbase[-1] + m * crows[-1]
                    + (gb[-1] - gb[-2]) * (GRP // 2))
        rows = np.full(TT * TPE, zrowp, dtype=np.int64)
        rows[sl] = pairrow[src_s[sel]]
        idxps.append(pack_idx(rows.astype(np.int16)))

        # metac: field-major [da, c_b x4]
        f1 = np.zeros((TT * TPE, 5), dtype=np.float32)
        f2 = np.zeros((TT * TPE, 5), dtype=np.float32)
        f1[:, 0] = 99.0
        f2[:, 0] = 99.0
        f1[sl, 0] = doff[sel]
        f2[sl, 0] = doff[sel]
        f1[sl, 1:] = c1[sel]
        f2[sl, 1:] = c2[sel]
        metac1s.append(fieldize(f1, 5))
        metac2s.append(fieldize(f2, 5))

        # per-position layer-1 root term, transposed [D, NPC]
        nid = np.nonzero(own == m)[0]
        p = pos[nid]
        rt = np.zeros((NPC, D), dtype=np.float32)
        rt[p] = rootfull[nid]
        roots.append(np.ascontiguousarray(rt.T))
        node_ids.append(nid)
        node_pos.append(p)

    return dict(NPC=NPC, SEG=SEG, NTAB=NTAB, NBLK=NBLK, NGRP=NGRP,
                NPAGE=NPAGE, TT=TT, tile_blocks=tile_blocks,
                gb=gb, crows=crows, cbase=cbase,
                xedge=xedges, idxp=idxps, metac1=metac1s, metac2=metac2s,
                roottermT=roots, node_ids=node_ids, node_pos=node_pos)


def _woh_chunk(nc, wohp, iota_t, metat, c):
    """Build the weighted-one-hot rhs for CHK tiles of chunk c."""
    oh8 = wohp.tile([TPE, CHK * BLK], BF16, tag="oh")
    nc.vector.tensor_tensor(
        out=_expand(oh8[:], [[BLK, CHK], [1, BLK]]),
        in0=iota_t,
        in1=_expand(metat[:], [[1, CHK], [0, BLK]], col_offset=c * CHK),
        op=ALU.is_equal,
    )
    wohF = wohp.tile([TPE, CHK * 4 * BLK], BF16, tag="wohF")
    nc.vector.tensor_tensor(
        out=_expand(wohF[:], [[4 * BLK, CHK], [BLK, 4], [1, BLK]]),
        in0=_expand(oh8[:], [[BLK, CHK], [0, 4], [1, BLK]]),
        in1=_expand(metat[:], [[1, CHK], [G, 4], [0, BLK]],
                    col_offset=G + c * CHK),
        op=ALU.mult,
    )
    return wohF


def _flush_to_sbig4(nc, gq, psum_blk, sbig4):
    """psum [D, 4*GRP] (block-major) -> sbig4 cols (basis-major, quad-wide)."""
    for b in range(4):
        nc.scalar.copy(
            out=_expand(sbig4[:], [[BLK, 4], [1, BLK]],
                        col_offset=b * QUAD * GRP + gq * GRP),
            in_=_expand(psum_blk[:], [[4 * BLK, 4], [1, BLK]],
                        col_offset=b * BLK),
        )


def _layer1(tc, nc, pools, prm, D, xedge, metap, iota_t, bas_t,
            roottermT, hTbuf, hsl_chunks):
    """Generator: yields ('cc', chunk) after a chunk's last table write and
    ('prep', k) slots where gather desc-gen prefetch should be emitted."""
    meta, xp, gath, wohp, hp, sb4p, psp = pools
    NPAGE, tile_blocks = prm["NPAGE"], prm["tile_blocks"]
    NGRP, gb = prm["NGRP"], prm["gb"]

    def chunk_of(g):
        for c in range(NCHUNK):
            if g < gb[c + 1]:
                return c
        raise AssertionError

    def emit_tail(p):
        """Deferred tail for page p (groups 4p..4p+3)."""
        g0 = p * QUAD
        sbig4 = sbig4s.pop(p)
        pquad = psp.tile([D, QUAD * GRP], F32, tag="quad", bufs=2)
        for b in range(4):
            nc.tensor.matmul(
                out=pquad[:],
                lhsT=bas_t[:, b * D:(b + 1) * D],
                rhs=sbig4[:, b * QUAD * GRP:(b + 1) * QUAD * GRP],
                start=(b == 0), stop=(b == 3))
        rtt = rtts.pop(p)
        hq = hp.tile([D, QUAD * GRP], F32, tag="hq")
        nc.vector.tensor_tensor(out=hq[:], in0=pquad[:], in1=rtt[:],
                                op=ALU.add)
        nc.scalar.activation(
            out=hTbuf[0:D, g0 * GRP:(g0 + QUAD) * GRP],
            in_=hq[:], func=AF.Relu)
        for gg in range(g0, g0 + QUAD):
            htr = hp.tile([GRP, DW], BF16, tag="htr")
            nc.scalar.dma_start_transpose(
                out=htr[:], in_=hTbuf[:, gg * GRP:(gg + 1) * GRP])
            ck = chunk_of(gg)
            hc = hsl_chunks[ck]
            grel = gg - gb[ck]
            base = hc[:]
            dstap = bass.AP(
                base.tensor,
                base.offset + grel * (GRP // 2) * (2 * DW),
                [[2 * DW, GRP // 2], [DW, 2], [1, DW]])
            nc.scalar.dma_start(out=dstap, in_=htr[:])

    sbig4s = {}
    rtts = {}
    tcount = 0
    pending = None
    psum_blks = {}
    for q in range(NPAGE):
        metat = meta.tile([TPE, 5 * G], BF16, tag="metat1")
        nc.sync.dma_start(out=metat[:], in_=metap[q])
        xpage = xp.tile([TPE, G * DW], BF16, tag="xpage")
        nc.sync.dma_start(out=xpage[:], in_=xedge[q])
        sbig4s[q] = sb4p.tile([D, 4 * QUAD * GRP], BF16, tag="sbig4", name="sbig4")
        rtts[q] = hp.tile([D, QUAD * GRP], F32, tag="rtt", name="rtt")
        nc.sync.dma_start(
            out=rtts[q][:],
            in_=roottermT[:, q * QUAD * GRP:(q + 1) * QUAD * GRP])
        for c in range(G // CHK):
            wohF = _woh_chunk(nc, wohp, iota_t, metat, c)
            if pending is not None and c == 2:
                p = pending
                pending = None
                emit_tail(p)
                gend = (p + 1) * QUAD - 1
                ck = chunk_of(gend)
                if gend == gb[ck + 1] - 1:
                    yield ('cc', ck)
            for u in range(CHK):
                blk, fst, lst = tile_blocks[tcount]
                g, bi = blk // 4, blk % 4
                gq = g % QUAD
                if fst:
                    if bi == 0:
                        psum_blks[g] = psp.tile([D, 4 * GRP], F32, tag="blk",
                                                bufs=4, name="pblk")
                    pb = psum_blks[g]
                else:
                    pb = psum_blks[g]
                nc.tensor.matmul(
                    out=pb[:, bi * 4 * BLK:(bi + 1) * 4 * BLK],
                    lhsT=xpage[:, (c * CHK + u) * DW:(c * CHK + u) * DW + D],
                    rhs=wohF[:, u * 4 * BLK:(u + 1) * 4 * BLK],
                    start=fst, stop=lst)
                if lst and bi == 3:
                    _flush_to_sbig4(nc, gq, psum_blks.pop(g), sbig4s[q])
                    if gq == QUAD - 1:
                        if DEFER_TAILS:
                            pending = q
                        else:
                            emit_tail(q)
                            gend = (q + 1) * QUAD - 1
                            ck = chunk_of(gend)
                            if gend == gb[ck + 1] - 1:
                                yield ('cc', ck)
                tcount += 1
    if pending is not None:
        p = pending
        emit_tail(p)
        gend = (p + 1) * QUAD - 1
        ck = chunk_of(gend)
        if gend == gb[ck + 1] - 1:
            yield ('cc', ck)


def _layer2(tc, nc, pools, prm, D, idxall, metap, preps, iota_t, bas_t,
            rt_t, hTbuf, outT, emit_prep):
    meta, xp, gath, wohp, hp, sb4p, psp = pools
    NPAGE, tile_blocks = prm["NPAGE"], prm["tile_blocks"]

    tcount = 0
    psum_blks = {}
    for q in range(NPAGE):
        if USE_PREP_TRIGGER:
            if q == 0:
                nc.gpsimd.trigger_dma(count=None)  # fire prefetched pages
            elif q - 1 + PREFETCH < NPAGE:
                emit_prep(q - 1 + PREFETCH)
                nc.gpsimd.trigger_dma(count=None)
            gbuf = preps[q]
        else:
            gbuf = emit_prep(q)
        metat = meta.tile([TPE, 5 * G], BF16, tag="metat2")
        nc.sync.dma_start(out=metat[:], in_=metap[q])
        sbig4 = sb4p.tile([D, 4 * QUAD * GRP], BF16, tag="sbig4")
        for c in range(G // CHK):
            wohF = _woh_chunk(nc, wohp, iota_t, metat, c)
            for u in range(CHK):
                blk, fst, lst = tile_blocks[tcount]
                g, bi = blk // 4, blk % 4
                gq = g % QUAD
                if fst and bi == 0:
                    psum_blks[g] = psp.tile([D, 4 * GRP], F32, tag="blk",
                                            bufs=4, name="pblk")
                pb = psum_blks[g]
                r = c * CHK + u
                nc.tensor.matmul(
                    out=pb[:, bi * 4 * BLK:(bi + 1) * 4 * BLK],
                    lhsT=gbuf[:, r % 2, r // 2, 0:D],
                    rhs=wohF[:, u * 4 * BLK:(u + 1) * 4 * BLK],
                    start=fst, stop=lst)
                if lst and bi == 3:
                    _flush_to_sbig4(nc, gq, psum_blks.pop(g), sbig4)
                    if gq == QUAD - 1:
                        g0 = q * QUAD
                        pquad = psp.tile([D, QUAD * GRP], F32, tag="quad",
                                         bufs=2)
                        for b in range(4):
                            nc.tensor.matmul(
                                out=pquad[:],
                                lhsT=bas_t[:, b * D:(b + 1) * D],
                                rhs=sbig4[:, b * QUAD * GRP:
                                          (b + 1) * QUAD * GRP],
                                start=(b == 0), stop=False)
                        for gg in range(QUAD):
                            # root2 augmented with a bias row; hTbuf row D
                            # is all-ones so bias2 is added per column
                            nc.tensor.matmul(
                                out=pquad[:, gg * GRP:(gg + 1) * GRP],
                                lhsT=rt_t,
                                rhs=hTbuf[0:D + 1,
                                          (g0 + gg) * GRP:(g0 + gg + 1) * GRP],
                                start=False, stop=(gg == QUAD - 1))
                        o3 = hp.tile([D, QUAD * GRP], F32, tag="o3")
                        nc.scalar.copy(out=o3[:], in_=pquad[:])
                        nc.sync.dma_start(
                            out=outT[:, g0 * GRP:(g0 + QUAD) * GRP],
                            in_=o3[:])
                tcount += 1


def _build(prm, D):
    NPC, SEG, NTAB = prm["NPC"], prm["SEG"], prm["NTAB"]
    NPAGE, NGRP, gb = prm["NPAGE"], prm["NGRP"], prm["gb"]
    nc = bacc.Bacc()

    xedge = nc.dram_tensor("xedge", [NPAGE, TPE, G * DW], BF16, kind="ExternalInput")
    idxp = nc.dram_tensor("idxp", [TPE, NPAGE * 2 * 128], I16, kind="ExternalInput")
    metac1 = nc.dram_tensor("metac1", [NPAGE, TPE, 5 * G], BF16, kind="ExternalInput")
    metac2 = nc.dram_tensor("metac2", [NPAGE, TPE, 5 * G], BF16, kind="ExternalInput")
    roottermT = nc.dram_tensor("roottermT", [D, NPC], F32, kind="ExternalInput")
    iota = nc.dram_tensor("iota", [TPE, CHK * BLK], BF16, kind="ExternalInput")
    bas1 = nc.dram_tensor("bas1", [D, 4 * D], BF16, kind="ExternalInput")
    bas2 = nc.dram_tensor("bas2", [D, 4 * D], BF16, kind="ExternalInput")
    rt2 = nc.dram_tensor("rt2", [D + 1, D], BF16, kind="ExternalInput")
    cpad = nc.dram_tensor("cpad", [TPE - D, NPC], BF16, kind="ExternalInput")
    outT = nc.dram_tensor("outT", [D, NPC], F32, kind="ExternalOutput")
    hdump = nc.dram_tensor("hdump", [TPE, NPC], BF16, kind="ExternalOutput")

    with tile.TileContext(nc) as tc:
        with (
            tc.tile_pool(name="const", bufs=1) as cst,
            tc.tile_pool(name="meta", bufs=3) as meta,
            tc.tile_pool(name="xp", bufs=2) as xp,
            tc.tile_pool(name="gath", bufs=3) as gath,
            tc.tile_pool(name="woh", bufs=3) as wohp,
            tc.tile_pool(name="hp", bufs=2) as hp,
            tc.tile_pool(name="sb4", bufs=2) as sb4p,
            tc.tile_pool(name="ps", bufs=1, space="PSUM") as psp,
            tc.tile_pool(name="dram", bufs=1, space="DRAM") as dramp,
        ):
            pools = (meta, xp, gath, wohp, hp, sb4p, psp)

            iota_t = cst.tile([TPE, CHK * BLK], BF16)
            nc.sync.dma_start(out=iota_t[:], in_=iota[:])
            bas1_t = cst.tile([D, 4 * D], BF16)
            nc.sync.dma_start(out=bas1_t[:], in_=bas1[:])
            bas2_t = cst.tile([D, 4 * D], BF16)
            nc.sync.dma_start(out=bas2_t[:], in_=bas2[:])
            rt2_t = cst.tile([D + 1, D], BF16)
            nc.sync.dma_start(out=rt2_t[:], in_=rt2[:])
            zed_t = cst.tile([GRP, 2 * DW], BF16)
            nc.gpsimd.memset(zed_t[:], 0.0)
            idxall = cst.tile([TPE, NPAGE * 2 * 128], I16)
            nc.sync.dma_start(out=idxall[:], in_=idxp[:])
            # hTbuf rows D..127: row D all-ones (bias row for the layer-2
            # root matmul), rest zeros -- DMAed from a host constant.
            hTbuf = cst.tile([TPE, NPC], BF16)
            nc.sync.dma_start(out=hTbuf[D:TPE, :], in_=cpad[:])

            # per-chunk local h slices (pair-row layout); last chunk carries
            # the zero tail rows (dummy gather target)
            hsl_chunks = []
            chunk_rows = []
            for ckk in range(NCHUNK):
                rows = (gb[ckk + 1] - gb[ckk]) * (GRP // 2)
                if ckk == NCHUNK - 1:
                    rows += (SEG - NPC) // 2
                t = dramp.tile([rows, 2 * DW], BF16, name=f"hslc{ckk}")
                hsl_chunks.append(t)
                chunk_rows.append(rows)

            hfull = dramp.tile([NTAB // 2, 2 * DW], BF16)

            # zero rows at the tail of the last chunk
            tail_rows = (SEG - NPC) // 2
            nc.sync.dma_start(
                out=hsl_chunks[-1][chunk_rows[-1] - tail_rows:chunk_rows[-1], :],
                in_=zed_t[0:tail_rows, :])

            dma_sem = nc.alloc_semaphore("swdge_gather")
            preps = {}

            def emit_prep(p):
                """Two half-row gathers for page p: even-parity tiles read
                the even 256-B half (elem_step=512 B), odd tiles the odd."""
                gbuf = gath.tile([TPE, 2, G // 2, DW], BF16, tag="gbuf")
                preps[p] = gbuf
                kw = {}
                if USE_PREP_TRIGGER:
                    kw = dict(prepare_only=True, sem=dma_sem)
                for e in range(2):
                    nc.gpsimd.dma_gather(
                        out_ap=gbuf[:, e, :, :],
                        in_ap=hfull[:, DW * e:DW * (e + 1)],
                        idxs_ap=idxall[:, (p * 2 + e) * 128:
                                       (p * 2 + e + 1) * 128],
                        num_idxs=(G // 2) * TPE, num_idxs_reg=(G // 2) * TPE,
                        elem_size=DW, elem_step=2 * DW,
                        single_packet=False, **kw)
                return gbuf

            cbase = prm["cbase"]
            nprep = 0
            if USE_PREP_TRIGGER:
                for k in range(min(PREPS_BEFORE_CC[0], NPAGE - nprep)):
                    emit_prep(nprep)
                    nprep += 1
            ncc = 0
            for kind, ck in _layer1(tc, nc, pools, prm, D, xedge, metac1,
                                    iota_t[:], bas1_t, roottermT, hTbuf,
                                    hsl_chunks):
                assert kind == 'cc'
                rows = chunk_rows[ck]
                nc.gpsimd.collective_compute(
                    "AllGather",
                    ALU.bypass,
                    replica_groups=[list(range(M))],
                    ins=[hsl_chunks[ck][:]],
                    outs=[hfull[cbase[ck]:cbase[ck] + M * rows, :]],
                )
                ncc += 1
                if USE_PREP_TRIGGER and ncc < NCHUNK:
                    budget = PREPS_BEFORE_CC[ncc]
                    for k in range(min(budget,
                                       min(PREFETCH, NPAGE) - nprep)):
                        emit_prep(nprep)
                        nprep += 1
            while USE_PREP_TRIGGER and nprep < min(PREFETCH, NPAGE):
                emit_prep(nprep)
                nprep += 1

            _layer2(tc, nc, pools, prm, D, idxall, metac2, preps,
                    iota_t[:], bas2_t, rt2_t[:], hTbuf, outT, emit_prep)
            nc.sync.dma_start(out=hdump[:], in_=hTbuf[:])
    nc.compile()
    return nc


def kernel(entity, edge_index, edge_attr, edge_type, edge_norm,
           basis1, att1, root1, bias1, basis2, att2, root2, bias2):
    N, D = entity.shape
    entity = np.asarray(entity, dtype=np.float32)
    prm = _prep(N, D, np.asarray(edge_index), np.asarray(edge_type),
                np.asarray(edge_norm), np.asarray(att1), np.asarray(att2),
                entity, np.asarray(root1), np.asarray(bias1))
    NPC = prm["NPC"]

    iota_arr = np.tile(np.arange(BLK, dtype=np.float32), (TPE, CHK)).astype(BF)
    b1 = np.ascontiguousarray(
        np.asarray(basis1, np.float32).transpose(1, 0, 2).reshape(D, 4 * D)).astype(BF)
    b2 = np.ascontiguousarray(
        np.asarray(basis2, np.float32).transpose(1, 0, 2).reshape(D, 4 * D)).astype(BF)

    nc = _build(prm, D)

    cpad_arr = np.zeros((TPE - D, NPC), dtype=BF)
    cpad_arr[0, :] = 1.0

    in_maps = []
    for m in range(M):
        in_maps.append({
            "xedge": prm["xedge"][m],
            "idxp": prm["idxp"][m],
            "metac1": prm["metac1"][m],
            "metac2": prm["metac2"][m],
            "roottermT": prm["roottermT"][m],
            "iota": iota_arr,
            "bas1": b1,
            "bas2": b2,
            "rt2": np.concatenate(
                [np.asarray(root2, np.float32),
                 np.asarray(bias2, np.float32).reshape(1, D)]).astype(BF),
            "cpad": cpad_arr,
        })
    kwargs = {}
    if TRACE:
        kwargs = dict(trace=True, tmpdir=TRACE_DIR)
    res = run_bass_kernel_spmd(nc, in_maps, core_ids=list(range(M)), **kwargs)
    global LAST
    LAST = res
    out = np.empty((N, D), dtype=np.float32)
    for m in range(M):
        o = np.ascontiguousarray(res.results[m]["outT"].T)
        out[prm["node_ids"][m]] = o[prm["node_pos"][m]]
    return np.ascontiguousarray(out)


LAST = None
TRACE = False
TRACE_DIR = None
